# revision 4
# baseline (speedup 1.0000x reference)
"""Trainium2 Bass kernel for a dense transformer block (RMSNorm -> causal MHA
-> residual -> RMSNorm -> SwiGLU MLP -> residual), distributed over 8
NeuronCores with zero collectives.

Sharding: core c handles batch b = c//2 and query-token half  half = c%2.
Each core computes K/V for its whole batch (replicated within the pair), and
queries / out-proj / MLP only for its 1024 tokens.  All activations are kept
in transposed [feature, token] layout on device so no on-device transposes are
needed anywhere; matmuls run in float32r (full-speed fp32 mode on the PE).

kernel(**inputs) takes the full unsharded inputs and returns the full output.
"""

import numpy as np

import concourse.bass as bass
import concourse.bacc as bacc
import concourse.mybir as mybir
from concourse.tile import TileContext
from concourse.bass_utils import run_bass_kernel_spmd

F32 = mybir.dt.float32
F32R = mybir.dt.float32r
AF = mybir.ActivationFunctionType
ALU = mybir.AluOpType

P = 128
N_CORES = 8
EPS = 1e-6


class CFG:
    def __init__(self, D, T, TD, FF, QT, NS):
        self.D, self.T, self.TD, self.FF, self.QT, self.NS = D, T, TD, FF, QT, NS
        self.DT = D // P          # d-tiles (contraction tiles over model dim)
        self.H = TD // P          # heads
        self.KT = T // P          # key tiles
        self.NQS = QT // NS       # query slices
        self.NBLK = T // NS       # kv token blocks (norm+proj granularity)
        self.QBLK = QT // NS      # query token blocks
        self.NVS = TD // NS       # v column slabs
        self.NFT = FF // P        # total ff tiles
        self.FT_SP = 4            # ff tiles per super-block
        self.NSP = self.NFT // self.FT_SP
        self.NDCT = D // P        # output col tiles
        self.ISQ = 1.0 / float(np.sqrt(P))

    def nkt(self, qs):
        # causal: query slice qs (local) may attend up to k < qs*NS + NS + QT
        return min((qs * self.NS + self.NS + self.QT) // P, self.KT)


FULL = CFG(D=2048, T=2048, TD=2048, FF=8192, QT=1024, NS=512)


def build(cfg):
    D, T, TD, FF, QT, NS = cfg.D, cfg.T, cfg.TD, cfg.FF, cfg.QT, cfg.NS
    DT, H, KT = cfg.DT, cfg.H, cfg.KT

    nc = bacc.Bacc("TRN2", target_bir_lowering=False, num_devices=N_CORES)

    # ---- inputs (pre-tiled on host) ----
    xT_t = nc.dram_tensor("xT_t", [DT, P, T], F32, kind="ExternalInput")
    ones_in = nc.dram_tensor("ones_in", [P, 1], F32R, kind="ExternalInput")
    xTq_t = nc.dram_tensor("xTq_t", [DT, P, QT], F32, kind="ExternalInput")
    mask_t = nc.dram_tensor("mask_t", [P, KT, QT], F32, kind="ExternalInput")
    wq_t = nc.dram_tensor("wq_t", [TD // P, P, DT, P], F32R, kind="ExternalInput")
    wk_t = nc.dram_tensor("wk_t", [TD // P, P, DT, P], F32R, kind="ExternalInput")
    wv_t = nc.dram_tensor("wv_t", [cfg.NVS, P, DT, NS], F32R, kind="ExternalInput")
    wo_t = nc.dram_tensor("wo_t", [cfg.NDCT, P, TD // P, P], F32R, kind="ExternalInput")
    wg_t = nc.dram_tensor("wg_t", [cfg.NFT, P, DT, P], F32R, kind="ExternalInput")
    wu_t = nc.dram_tensor("wu_t", [cfg.NFT, P, DT, P], F32R, kind="ExternalInput")
    wd_t = nc.dram_tensor(
        "wd_t", [cfg.NSP, cfg.NDCT, P, cfg.FT_SP, P], F32R, kind="ExternalInput"
    )
    yT = nc.dram_tensor("yT", [D, QT], F32, kind="ExternalOutput")

    # ---- scratch DRAM ----
    kT_d = nc.dram_tensor("kT_d", [TD, T], F32R)
    qT_d = nc.dram_tensor("qT_d", [TD, QT], F32R)
    v_d = nc.dram_tensor("v_d", [KT, P, H, P], F32R)
    oT_d = nc.dram_tensor("oT_d", [TD, QT], F32R)
    x2T_d = nc.dram_tensor("x2T_d", [DT, P, QT], F32)

    with TileContext(nc) as tc:
        with tc.tile_pool(name="const", bufs=1) as cpool:
            ones = cpool.tile([P, 1], F32R, tag="ones")
            nc.sync.dma_start(out=ones[:], in_=ones_in[:])
            epsT = cpool.tile([1, 1], F32, tag="eps")
            nc.vector.memset(epsT[:], EPS)
            rec3 = cpool.tile([1, QT], F32, tag="rec3")
            bc2 = cpool.tile([P, QT], F32, tag="bc2")

            # ============ P1: rmsnorm + qkv projections ============
            with tc.tile_pool(name="p1", bufs=3) as p1, \
                 tc.tile_pool(name="p1w", bufs=2) as p1w, \
                 tc.tile_pool(name="hpool", bufs=2) as hpool, \
                 tc.tile_pool(name="p1ps", bufs=2, space="PSUM") as p1ps, \
                 tc.tile_pool(name="p1psg", bufs=3, space="PSUM") as p1psg:

                def norm_block(src_t, tok0):
                    """Normalize NS tokens from src_t starting at tok0.
                    Returns resident hT tile [P, DT, NS] (fp32)."""
                    ssp = p1ps.tile([1, NS], F32, tag="ssum")
                    for dt in range(DT):
                        xt = p1.tile([P, NS], F32, tag="xt")
                        nc.sync.dma_start(
                            out=xt[:], in_=src_t[dt, :, tok0:tok0 + NS])
                        sq = p1.tile([P, NS], F32R, tag="sq")
                        nc.scalar.activation(sq[:], xt[:], AF.Square)
                        nc.tensor.matmul(ssp[:], (ones[:]), (sq[:]),
                                         start=(dt == 0), stop=(dt == DT - 1))
                    srow = p1.tile([1, NS], F32, tag="srow")
                    nc.scalar.activation(srow[:], ssp[:], AF.Sqrt,
                                         scale=1.0 / D, bias=epsT[:])
                    rec = p1.tile([1, NS], F32, tag="rec")
                    nc.vector.reciprocal(rec[:], srow[:])
                    bc = p1.tile([P, NS], F32, tag="bc")
                    nc.gpsimd.partition_broadcast(bc[:], rec[:1, :])
                    hT = hpool.tile([P, DT, NS], F32R, tag="hT")
                    for dt in range(DT):
                        xt = p1.tile([P, NS], F32, tag="xtb")
                        nc.sync.dma_start(
                            out=xt[:], in_=src_t[dt, :, tok0:tok0 + NS])
                        nc.vector.tensor_tensor(hT[:, dt, :], xt[:], bc[:],
                                                ALU.mult)
                    return hT

                # ---- kv blocks over the full batch sequence ----
                for tb in range(cfg.NBLK):
                    tok0 = tb * NS
                    hT = norm_block(xT_t, tok0)
                    # kT projection [TD, NS-chunk]
                    for ct in range(TD // P):
                        wk = p1w.tile([P, DT, P], F32R, tag="wk")
                        nc.sync.dma_start(out=wk[:], in_=wk_t[ct])
                        kps = p1psg.tile([P, NS], F32, tag="gps")
                        for dt in range(DT):
                            nc.tensor.matmul(kps[:], (wk[:, dt, :]),
                                             (hT[:, dt, :]),
                                             start=(dt == 0),
                                             stop=(dt == DT - 1))
                        kcp = p1.tile([P, NS], F32R, tag="kcp")
                        nc.scalar.copy(kcp[:], kps[:])
                        nc.sync.dma_start(
                            out=kT_d[ct * P:(ct + 1) * P, tok0:tok0 + NS],
                            in_=kcp[:])
                    # v projection (natural layout, tiled DRAM)
                    for vs in range(cfg.NVS):
                        wv = p1w.tile([P, DT, NS], F32R, tag="wv")
                        nc.sync.dma_start(out=wv[:], in_=wv_t[vs])
                        hpp = NS // P  # heads per v-slab
                        for tt in range(NS // P):
                            kt = (tok0 // P) + tt
                            vps = p1psg.tile([P, NS], F32, tag="gps")
                            for dt in range(DT):
                                nc.tensor.matmul(
                                    vps[:],
                                    (hT[:, dt, tt * P:(tt + 1) * P]),
                                    (wv[:, dt, :]),
                                    start=(dt == 0), stop=(dt == DT - 1))
                            vcp = p1.tile([P, NS], F32R, tag="vcp")
                            nc.scalar.copy(vcp[:], vps[:])
                            nc.sync.dma_start(
                                out=v_d[kt, :, vs * hpp:(vs + 1) * hpp, :],
                                in_=vcp[:].rearrange("p (a c) -> p a c", c=P))
                # ---- query blocks ----
                for qb in range(cfg.QBLK):
                    tok0 = qb * NS
                    hTq = norm_block(xTq_t, tok0)
                    for ct in range(TD // P):
                        wq = p1w.tile([P, DT, P], F32R, tag="wk")
                        nc.sync.dma_start(out=wq[:], in_=wq_t[ct])
                        qps = p1psg.tile([P, NS], F32, tag="gps")
                        for dt in range(DT):
                            nc.tensor.matmul(qps[:], (wq[:, dt, :]),
                                             (hTq[:, dt, :]),
                                             start=(dt == 0),
                                             stop=(dt == DT - 1))
                        qcp = p1.tile([P, NS], F32R, tag="kcp")
                        nc.scalar.copy(qcp[:], qps[:])
                        nc.sync.dma_start(
                            out=qT_d[ct * P:(ct + 1) * P, tok0:tok0 + NS],
                            in_=qcp[:])

            # ============ P2: causal attention ============
            with tc.tile_pool(name="p2", bufs=3) as p2, \
                 tc.tile_pool(name="p2h", bufs=2) as p2h, \
                 tc.tile_pool(name="p2m", bufs=1) as p2m, \
                 tc.tile_pool(name="p2ps", bufs=3, space="PSUM") as p2ps, \
                 tc.tile_pool(name="p2acc", bufs=2, space="PSUM") as p2acc:
                mask_sb = p2m.tile([P, KT, QT], F32, tag="mask")
                nc.sync.dma_start(out=mask_sb[:], in_=mask_t[:])
                for h in range(H):
                    kh = p2h.tile([P, T], F32R, tag="kh")
                    nc.sync.dma_start(out=kh[:], in_=kT_d[h * P:(h + 1) * P, :])
                    qh = p2h.tile([P, QT], F32R, tag="qh")
                    nc.sync.dma_start(out=qh[:], in_=qT_d[h * P:(h + 1) * P, :])
                    vh = p2h.tile([P, KT, P], F32R, tag="vh")
                    nc.sync.dma_start(
                        out=vh[:],
                        in_=v_d[:, :, h, :].rearrange("a p c -> p a c"))
                    for qs in range(cfg.NQS):
                        nkt = cfg.nkt(qs)
                        oacc = p2acc.tile([P, NS], F32, tag="oacc")
                        dacc = p2acc.tile([1, NS], F32, tag="dacc")
                        for kt in range(nkt):
                            scp = p2ps.tile([P, NS], F32, tag="scp")
                            nc.tensor.matmul(
                                scp[:], (kh[:, kt * P:(kt + 1) * P]),
                                (qh[:, qs * NS:(qs + 1) * NS]),
                                start=True, stop=True)
                            pt = p2.tile([P, NS], F32, tag="pt")
                            nc.vector.scalar_tensor_tensor(
                                pt[:], scp[:], cfg.ISQ,
                                mask_sb[:, kt, qs * NS:(qs + 1) * NS],
                                ALU.mult, ALU.add)
                            pex = p2.tile([P, NS], F32R, tag="pex")
                            nc.scalar.activation(pex[:], pt[:], AF.Exp)
                            nc.tensor.matmul(dacc[:], (ones[:]), (pex[:]),
                                             start=(kt == 0),
                                             stop=(kt == nkt - 1))
                            nc.tensor.matmul(oacc[:], (vh[:, kt, :]),
                                             (pex[:]),
                                             start=(kt == 0),
                                             stop=(kt == nkt - 1))
                        recd = p2.tile([1, NS], F32, tag="recd")
                        nc.vector.reciprocal(recd[:], dacc[:])
                        rbc = p2.tile([P, NS], F32, tag="rbc")
                        nc.gpsimd.partition_broadcast(rbc[:], recd[:1, :])
                        ot = p2.tile([P, NS], F32R, tag="ot")
                        nc.vector.tensor_tensor(ot[:], oacc[:], rbc[:],
                                                ALU.mult)
                        nc.sync.dma_start(
                            out=oT_d[h * P:(h + 1) * P,
                                     qs * NS:(qs + 1) * NS],
                            in_=ot[:])

            # ============ P3: out-projection + residual + norm2 stats ========
            with tc.tile_pool(name="p3", bufs=3) as p3, \
                 tc.tile_pool(name="p3o", bufs=2) as p3o, \
                 tc.tile_pool(name="p3ps", bufs=2, space="PSUM") as p3ps, \
                 tc.tile_pool(name="p3ss", bufs=2, space="PSUM") as p3ss:
                for ts in range(cfg.NQS):
                    ot_sb = p3o.tile([P, TD // P, NS], F32R, tag="otsb")
                    nc.sync.dma_start(
                        out=ot_sb[:],
                        in_=oT_d[:, ts * NS:(ts + 1) * NS].rearrange(
                            "(a p) c -> p a c", p=P))
                    ssp2 = p3ss.tile([1, NS], F32, tag="ss2")
                    for dct in range(cfg.NDCT):
                        wo = p3.tile([P, TD // P, P], F32R, tag="wo")
                        nc.sync.dma_start(out=wo[:], in_=wo_t[dct])
                        ops = p3ps.tile([P, NS], F32, tag="ops")
                        for tdt in range(TD // P):
                            nc.tensor.matmul(ops[:], (wo[:, tdt, :]),
                                             (ot_sb[:, tdt, :]),
                                             start=(tdt == 0),
                                             stop=(tdt == TD // P - 1))
                        xq = p3.tile([P, NS], F32, tag="xq")
                        nc.sync.dma_start(
                            out=xq[:],
                            in_=xTq_t[dct, :, ts * NS:(ts + 1) * NS])
                        x2 = p3.tile([P, NS], F32, tag="x2")
                        nc.vector.tensor_tensor(x2[:], ops[:], xq[:], ALU.add)
                        nc.sync.dma_start(
                            out=x2T_d[dct, :, ts * NS:(ts + 1) * NS],
                            in_=x2[:])
                        sq2 = p3.tile([P, NS], F32R, tag="sq2")
                        nc.scalar.activation(sq2[:], x2[:], AF.Square)
                        nc.tensor.matmul(ssp2[:], (ones[:]), (sq2[:]),
                                         start=(dct == 0),
                                         stop=(dct == cfg.NDCT - 1))
                    srow2 = p3.tile([1, NS], F32, tag="sr2")
                    nc.scalar.activation(srow2[:], ssp2[:], AF.Sqrt,
                                         scale=1.0 / D, bias=epsT[:])
                    nc.vector.reciprocal(rec3[:, ts * NS:(ts + 1) * NS],
                                         srow2[:])
                nc.gpsimd.partition_broadcast(bc2[:], rec3[:1, :])

            # ============ P4+P5: h2 + SwiGLU MLP + residual ============
            with tc.tile_pool(name="pres", bufs=1) as pres, \
                 tc.tile_pool(name="p5", bufs=2) as p5, \
                 tc.tile_pool(name="p5s", bufs=2) as p5s, \
                 tc.tile_pool(name="p5ps", bufs=2, space="PSUM") as p5ps:
                h2T = pres.tile([P, DT, QT], F32R, tag="h2T")
                y_acc = pres.tile([P, DT, QT], F32, tag="y_acc")
                for dt in range(DT):
                    x2t = p5.tile([P, QT], F32, tag="x2t")
                    nc.sync.dma_start(out=x2t[:], in_=x2T_d[dt])
                    nc.vector.tensor_tensor(h2T[:, dt, :], x2t[:], bc2[:],
                                            ALU.mult)
                    nc.scalar.copy(y_acc[:, dt, :], x2t[:])
                for sp in range(cfg.NSP):
                    mt = p5s.tile([P, cfg.FT_SP, QT], F32R, tag="mt", bufs=1)
                    for ft in range(cfg.FT_SP):
                        gft = sp * cfg.FT_SP + ft
                        wg = p5.tile([P, DT, P], F32R, tag="wg")
                        nc.sync.dma_start(out=wg[:], in_=wg_t[gft])
                        wu = p5.tile([P, DT, P], F32R, tag="wu")
                        nc.sync.dma_start(out=wu[:], in_=wu_t[gft])
                        for ws in range(cfg.NQS):
                            gps = p5ps.tile([P, NS], F32, tag="gps")
                            for dt in range(DT):
                                nc.tensor.matmul(
                                    gps[:], (wg[:, dt, :]),
                                    (h2T[:, dt, ws * NS:(ws + 1) * NS]),
                                    start=(dt == 0), stop=(dt == DT - 1))
                            ups = p5ps.tile([P, NS], F32, tag="ups")
                            for dt in range(DT):
                                nc.tensor.matmul(
                                    ups[:], (wu[:, dt, :]),
                                    (h2T[:, dt, ws * NS:(ws + 1) * NS]),
                                    start=(dt == 0), stop=(dt == DT - 1))
                            sg = p5.tile([P, NS], F32, tag="sg")
                            nc.scalar.activation(sg[:], gps[:], AF.Silu)
                            nc.vector.tensor_tensor(
                                mt[:, ft, ws * NS:(ws + 1) * NS], sg[:],
                                ups[:], ALU.mult)
                    for dct in range(cfg.NDCT):
                        wd = p5.tile([P, cfg.FT_SP, P], F32R, tag="wd")
                        nc.sync.dma_start(out=wd[:], in_=wd_t[sp, dct])
                        for ws in range(cfg.NQS):
                            dps = p5ps.tile([P, NS], F32, tag="dps")
                            for ft in range(cfg.FT_SP):
                                nc.tensor.matmul(
                                    dps[:], (wd[:, ft, :]),
                                    (mt[:, ft, ws * NS:(ws + 1) * NS]),
                                    start=(ft == 0),
                                    stop=(ft == cfg.FT_SP - 1))
                            ya = y_acc[:, dct, ws * NS:(ws + 1) * NS]
                            nc.vector.tensor_tensor(ya, ya, dps[:], ALU.add)
                for dt in range(DT):
                    nc.sync.dma_start(out=yT[dt * P:(dt + 1) * P, :],
                                      in_=y_acc[:, dt, :])

    nc.compile()
    return nc


# --------------------------------------------------------------------------
# Host side
# --------------------------------------------------------------------------

_NC_CACHE = {}


def _get_nc(cfg):
    key = (cfg.D, cfg.T, cfg.TD, cfg.FF, cfg.QT, cfg.NS)
    if key not in _NC_CACHE:
        _NC_CACHE[key] = build(cfg)
    return _NC_CACHE[key]


def prep_weights(cfg, w_qkv, w_out, w_gate, w_up, w_down, ln1, ln2):
    D, TD, FF, NS = cfg.D, cfg.TD, cfg.FF, cfg.NS
    DT = cfg.DT
    f32 = np.float32
    w_qkv_f = (np.asarray(w_qkv, f32) * np.asarray(ln1, f32)[None, :])
    wqT = w_qkv_f[0:TD].T
    wkT = w_qkv_f[TD:2 * TD].T
    wvT = w_qkv_f[2 * TD:3 * TD].T
    woT = np.asarray(w_out, f32).T            # [TD, D]
    wgT = (np.asarray(w_gate, f32) * np.asarray(ln2, f32)[None, :]).T
    wuT = (np.asarray(w_up, f32) * np.asarray(ln2, f32)[None, :]).T
    wdT = np.asarray(w_down, f32).T           # [FF, D]

    def tile_lhs(a, ncols):  # [D, C] -> [C/ncols? ...] lhsT tiles [ct, P, dt, cP]
        d, c = a.shape
        return np.ascontiguousarray(
            a.reshape(d // P, P, c // ncols, ncols).transpose(2, 1, 0, 3))

    wq_t = tile_lhs(wqT, P)
    wk_t = tile_lhs(wkT, P)
    wv_t = tile_lhs(wvT, NS)
    wo_t = tile_lhs(woT, P)
    wg_t = tile_lhs(wgT, P)
    wu_t = tile_lhs(wuT, P)
    # w_down: [FF, D] -> [sp, dct, P, ft, P]
    wd_t = np.ascontiguousarray(
        wdT.reshape(cfg.NSP, cfg.FT_SP, P, D // P, P).transpose(0, 3, 2, 1, 4))
    return dict(wq_t=wq_t, wk_t=wk_t, wv_t=wv_t, wo_t=wo_t, wg_t=wg_t,
                wu_t=wu_t, wd_t=wd_t)


def prep_core_inputs(cfg, xb, half, wdict):
    """Per-core tensors for batch slice xb [T, D] and query half."""
    T, D, QT, KT = cfg.T, cfg.D, cfg.QT, cfg.KT
    f32 = np.float32
    xT = np.ascontiguousarray(np.asarray(xb, f32).T)        # [D, T]
    qoff = half * QT
    xT_t = xT.reshape(cfg.DT, P, T)
    xTq_t = np.ascontiguousarray(xT[:, qoff:qoff + QT]).reshape(cfg.DT, P, QT)
    kk = np.arange(T)[:, None]
    qq = np.arange(QT)[None, :] + qoff
    m = np.where(kk <= qq, 0.0, -1e30).astype(f32)          # [T, QT]
    mask_t = np.ascontiguousarray(m.reshape(KT, P, QT).transpose(1, 0, 2))
    out = dict(xT_t=xT_t, xTq_t=xTq_t, mask_t=mask_t,
               ones_in=np.ones((P, 1), np.float32))
    out.update(wdict)
    return out


def run(cfg, x, w_qkv, w_out, w_gate, w_up, w_down, ln1, ln2):
    nc = _get_nc(cfg)
    wdict = prep_weights(cfg, w_qkv, w_out, w_gate, w_up, w_down, ln1, ln2)
    x = np.asarray(x, np.float32)
    Bc = x.shape[0]
    in_maps = []
    for c in range(N_CORES):
        b, half = divmod(c, 2)
        b = b % Bc
        in_maps.append(prep_core_inputs(cfg, x[b], half, wdict))
    res = run_bass_kernel_spmd(nc, in_maps, list(range(N_CORES)))
    y = np.empty((Bc, cfg.T, cfg.D), np.float32)
    for c in range(N_CORES):
        b, half = divmod(c, 2)
        if b < Bc:
            y[b, half * cfg.QT:(half + 1) * cfg.QT, :] = res.results[c]["yT"].T
    return y


def kernel(x, w_qkv, w_out, w_gate, w_up, w_down, ln1, ln2):
    return run(FULL, x, w_qkv, w_out, w_gate, w_up, w_down, ln1, ln2)


# revision 5
# speedup vs baseline: 28.5832x; 28.5832x over previous
"""Trainium2 Bass kernel for a dense transformer block (RMSNorm -> causal MHA
-> residual -> RMSNorm -> SwiGLU MLP -> residual), distributed over 8
NeuronCores with zero collectives.

Sharding: core c handles batch b = c//2 and query-token half  half = c%2.
Each core computes K/V for its whole batch (replicated within the pair), and
queries / out-proj / MLP only for its 1024 tokens.  All activations are kept
in transposed [feature, token] layout on device so no on-device transposes are
needed anywhere; matmuls run in float32r (full-speed fp32 mode on the PE).

kernel(**inputs) takes the full unsharded inputs and returns the full output.
"""

import numpy as np

import concourse.bass as bass
import concourse.bacc as bacc
import concourse.mybir as mybir
from concourse.tile import TileContext
from concourse.bass_utils import run_bass_kernel_spmd

F32 = mybir.dt.float32
F32R = mybir.dt.float32r
AF = mybir.ActivationFunctionType
ALU = mybir.AluOpType

P = 128
N_CORES = 8
EPS = 1e-6


class CFG:
    def __init__(self, D, T, TD, FF, QT, NS):
        self.D, self.T, self.TD, self.FF, self.QT, self.NS = D, T, TD, FF, QT, NS
        self.DT = D // P          # d-tiles (contraction tiles over model dim)
        self.H = TD // P          # heads
        self.KT = T // P          # key tiles
        self.NQS = QT // NS       # query slices
        self.NBLK = T // NS       # kv token blocks (norm+proj granularity)
        self.QBLK = QT // NS      # query token blocks
        self.NVS = TD // NS       # v column slabs
        self.NFT = FF // P        # total ff tiles
        self.FT_SP = 4            # ff tiles per super-block
        self.NSP = self.NFT // self.FT_SP
        self.NDCT = D // P        # output col tiles
        self.ISQ = 1.0 / float(np.sqrt(P))

    def nkt(self, qs):
        # interleaved queries: slot j is global token stride*j + parity, so
        # query slice qs may attend up to k <= (qs*NS + NS - 1)*stride + 1
        stride = self.T // self.QT
        return min((qs * self.NS + self.NS) * stride // P, self.KT)

    def mask_free(self, qs, kt):
        # tile fully allowed for every parity: k_max < stride * q_min
        stride = self.T // self.QT
        return (kt * P + P - 1) < stride * (qs * self.NS)


FULL = CFG(D=2048, T=2048, TD=2048, FF=8192, QT=1024, NS=512)


def build(cfg):
    D, T, TD, FF, QT, NS = cfg.D, cfg.T, cfg.TD, cfg.FF, cfg.QT, cfg.NS
    DT, H, KT = cfg.DT, cfg.H, cfg.KT

    nc = bacc.Bacc("TRN2", target_bir_lowering=False, num_devices=N_CORES)

    # ---- inputs (pre-tiled on host) ----
    xT_t = nc.dram_tensor("xT_t", [DT, P, T], F32, kind="ExternalInput")
    ones_in = nc.dram_tensor("ones_in", [P, 1], F32R, kind="ExternalInput")
    xTq_t = nc.dram_tensor("xTq_t", [DT, P, QT], F32, kind="ExternalInput")
    mask_t = nc.dram_tensor("mask_t", [P, KT, QT], F32, kind="ExternalInput")
    wq_t = nc.dram_tensor("wq_t", [TD // P, P, DT, P], F32R, kind="ExternalInput")
    wk_t = nc.dram_tensor("wk_t", [TD // P, P, DT, P], F32R, kind="ExternalInput")
    wv_t = nc.dram_tensor("wv_t", [cfg.NVS, P, DT, NS], F32R, kind="ExternalInput")
    wo_t = nc.dram_tensor("wo_t", [cfg.NDCT, P, TD // P, P], F32R, kind="ExternalInput")
    wg_t = nc.dram_tensor("wg_t", [cfg.NFT, P, DT, P], F32R, kind="ExternalInput")
    wu_t = nc.dram_tensor("wu_t", [cfg.NFT, P, DT, P], F32R, kind="ExternalInput")
    wd_t = nc.dram_tensor(
        "wd_t", [cfg.NSP, cfg.NDCT, P, cfg.FT_SP, P], F32R, kind="ExternalInput"
    )
    yT = nc.dram_tensor("yT", [D, QT], F32, kind="ExternalOutput")

    # ---- scratch DRAM ----
    kT_d = nc.dram_tensor("kT_d", [TD, T], F32R)
    qT_d = nc.dram_tensor("qT_d", [TD, QT], F32R)
    v_d = nc.dram_tensor("v_d", [H, KT, P, P], F32R)
    oT_d = nc.dram_tensor("oT_d", [TD, QT], F32R)
    x2T_d = nc.dram_tensor("x2T_d", [DT, P, QT], F32)

    with TileContext(nc) as tc:
        with tc.tile_pool(name="const", bufs=1) as cpool:
            ones = cpool.tile([P, 1], F32R, tag="ones")
            nc.sync.dma_start(out=ones[:], in_=ones_in[:])
            epsT = cpool.tile([1, 1], F32, tag="eps")
            nc.vector.memset(epsT[:], EPS)
            rec3 = cpool.tile([1, QT], F32, tag="rec3")
            bc2 = cpool.tile([P, QT], F32, tag="bc2")

            # ============ P1: rmsnorm + qkv projections ============
            with tc.tile_pool(name="p1", bufs=3) as p1, \
                 tc.tile_pool(name="p1w", bufs=2) as p1w, \
                 tc.tile_pool(name="hpool", bufs=2) as hpool, \
                 tc.tile_pool(name="p1ps", bufs=2, space="PSUM") as p1ps, \
                 tc.tile_pool(name="p1psg", bufs=3, space="PSUM") as p1psg:

                def norm_block(src_t, tok0):
                    """Normalize NS tokens from src_t starting at tok0.
                    Returns resident hT tile [P, DT, NS] (fp32)."""
                    ssp = p1ps.tile([1, NS], F32, tag="ssum")
                    for dt in range(DT):
                        xt = p1.tile([P, NS], F32, tag="xt")
                        nc.sync.dma_start(
                            out=xt[:], in_=src_t[dt, :, tok0:tok0 + NS])
                        sq = p1.tile([P, NS], F32R, tag="sq")
                        nc.scalar.activation(sq[:], xt[:], AF.Square)
                        nc.tensor.matmul(ssp[:], (ones[:]), (sq[:]),
                                         start=(dt == 0), stop=(dt == DT - 1))
                    srow = p1.tile([1, NS], F32, tag="srow")
                    nc.scalar.activation(srow[:], ssp[:], AF.Sqrt,
                                         scale=1.0 / D, bias=epsT[:])
                    rec = p1.tile([1, NS], F32, tag="rec")
                    nc.vector.reciprocal(rec[:], srow[:])
                    bc = p1.tile([P, NS], F32, tag="bc")
                    nc.gpsimd.partition_broadcast(bc[:], rec[:1, :])
                    hT = hpool.tile([P, DT, NS], F32R, tag="hT")
                    for dt in range(DT):
                        xt = p1.tile([P, NS], F32, tag="xtb")
                        nc.sync.dma_start(
                            out=xt[:], in_=src_t[dt, :, tok0:tok0 + NS])
                        nc.vector.tensor_tensor(hT[:, dt, :], xt[:], bc[:],
                                                ALU.mult)
                    return hT

                # ---- kv blocks over the full batch sequence ----
                for tb in range(cfg.NBLK):
                    tok0 = tb * NS
                    hT = norm_block(xT_t, tok0)
                    # kT projection [TD, NS-chunk]
                    for ct in range(TD // P):
                        wk = p1w.tile([P, DT, P], F32R, tag="wk")
                        nc.sync.dma_start(out=wk[:], in_=wk_t[ct])
                        kps = p1psg.tile([P, NS], F32, tag="gps")
                        for dt in range(DT):
                            nc.tensor.matmul(kps[:], (wk[:, dt, :]),
                                             (hT[:, dt, :]),
                                             start=(dt == 0),
                                             stop=(dt == DT - 1))
                        kcp = p1.tile([P, NS], F32R, tag="kcp")
                        nc.scalar.copy(kcp[:], kps[:])
                        nc.sync.dma_start(
                            out=kT_d[ct * P:(ct + 1) * P, tok0:tok0 + NS],
                            in_=kcp[:])
                    # v projection (natural layout, tiled DRAM)
                    for vs in range(cfg.NVS):
                        wv = p1w.tile([P, DT, NS], F32R, tag="wv")
                        nc.sync.dma_start(out=wv[:], in_=wv_t[vs])
                        hpp = NS // P  # heads per v-slab
                        for tt in range(NS // P):
                            kt = (tok0 // P) + tt
                            vps = p1psg.tile([P, NS], F32, tag="gps")
                            for dt in range(DT):
                                nc.tensor.matmul(
                                    vps[:],
                                    (hT[:, dt, tt * P:(tt + 1) * P]),
                                    (wv[:, dt, :]),
                                    start=(dt == 0), stop=(dt == DT - 1))
                            vcp = p1.tile([P, NS], F32R, tag="vcp")
                            nc.scalar.copy(vcp[:], vps[:])
                            for a in range(hpp):
                                nc.sync.dma_start(
                                    out=v_d[vs * hpp + a, kt, :, :],
                                    in_=vcp[:, a * P:(a + 1) * P])
                # ---- query blocks ----
                for qb in range(cfg.QBLK):
                    tok0 = qb * NS
                    hTq = norm_block(xTq_t, tok0)
                    for ct in range(TD // P):
                        wq = p1w.tile([P, DT, P], F32R, tag="wk")
                        nc.sync.dma_start(out=wq[:], in_=wq_t[ct])
                        qps = p1psg.tile([P, NS], F32, tag="gps")
                        for dt in range(DT):
                            nc.tensor.matmul(qps[:], (wq[:, dt, :]),
                                             (hTq[:, dt, :]),
                                             start=(dt == 0),
                                             stop=(dt == DT - 1))
                        qcp = p1.tile([P, NS], F32R, tag="kcp")
                        nc.scalar.copy(qcp[:], qps[:])
                        nc.sync.dma_start(
                            out=qT_d[ct * P:(ct + 1) * P, tok0:tok0 + NS],
                            in_=qcp[:])

            # ============ P2: causal attention ============
            with tc.tile_pool(name="p2", bufs=3) as p2, \
                 tc.tile_pool(name="p2h", bufs=2) as p2h, \
                 tc.tile_pool(name="p2m", bufs=1) as p2m, \
                 tc.tile_pool(name="p2ps", bufs=3, space="PSUM") as p2ps, \
                 tc.tile_pool(name="p2acc", bufs=2, space="PSUM") as p2acc:
                mask_sb = p2m.tile([P, KT, QT], F32, tag="mask")
                nc.sync.dma_start(out=mask_sb[:], in_=mask_t[:])
                for h in range(H):
                    kh = p2h.tile([P, T], F32R, tag="kh")
                    nc.sync.dma_start(out=kh[:], in_=kT_d[h * P:(h + 1) * P, :])
                    qh = p2h.tile([P, QT], F32R, tag="qh")
                    nc.sync.dma_start(out=qh[:], in_=qT_d[h * P:(h + 1) * P, :])
                    vh = p2h.tile([P, KT, P], F32R, tag="vh")
                    nc.sync.dma_start(
                        out=vh[:],
                        in_=v_d[h].rearrange("a p c -> p a c"))
                    for qs in range(cfg.NQS):
                        nkt = cfg.nkt(qs)
                        oacc = p2acc.tile([P, NS], F32, tag="oacc")
                        dacc = p2acc.tile([1, NS], F32, tag="dacc")
                        for kt in range(nkt):
                            scp = p2ps.tile([P, NS], F32, tag="scp")
                            nc.tensor.matmul(
                                scp[:], (kh[:, kt * P:(kt + 1) * P]),
                                (qh[:, qs * NS:(qs + 1) * NS]),
                                start=True, stop=True)
                            pex = p2.tile([P, NS], F32R, tag="pex")
                            if cfg.mask_free(qs, kt):
                                nc.scalar.activation(pex[:], scp[:], AF.Exp,
                                                     scale=cfg.ISQ)
                            else:
                                pt = p2.tile([P, NS], F32, tag="pt")
                                nc.vector.scalar_tensor_tensor(
                                    pt[:], scp[:], cfg.ISQ,
                                    mask_sb[:, kt, qs * NS:(qs + 1) * NS],
                                    ALU.mult, ALU.add)
                                nc.scalar.activation(pex[:], pt[:], AF.Exp)
                            nc.tensor.matmul(dacc[:], (ones[:]), (pex[:]),
                                             start=(kt == 0),
                                             stop=(kt == nkt - 1))
                            nc.tensor.matmul(oacc[:], (vh[:, kt, :]),
                                             (pex[:]),
                                             start=(kt == 0),
                                             stop=(kt == nkt - 1))
                        recd = p2.tile([1, NS], F32, tag="recd")
                        nc.vector.reciprocal(recd[:], dacc[:])
                        rbc = p2.tile([P, NS], F32, tag="rbc")
                        nc.gpsimd.partition_broadcast(rbc[:], recd[:1, :])
                        ot = p2.tile([P, NS], F32R, tag="ot")
                        nc.vector.tensor_tensor(ot[:], oacc[:], rbc[:],
                                                ALU.mult)
                        nc.sync.dma_start(
                            out=oT_d[h * P:(h + 1) * P,
                                     qs * NS:(qs + 1) * NS],
                            in_=ot[:])

            # ============ P3: out-projection + residual + norm2 stats ========
            with tc.tile_pool(name="p3", bufs=3) as p3, \
                 tc.tile_pool(name="p3o", bufs=2) as p3o, \
                 tc.tile_pool(name="p3ps", bufs=2, space="PSUM") as p3ps, \
                 tc.tile_pool(name="p3ss", bufs=2, space="PSUM") as p3ss:
                for ts in range(cfg.NQS):
                    ot_sb = p3o.tile([P, TD // P, NS], F32R, tag="otsb")
                    nc.sync.dma_start(
                        out=ot_sb[:],
                        in_=oT_d[:, ts * NS:(ts + 1) * NS].rearrange(
                            "(a p) c -> p a c", p=P))
                    ssp2 = p3ss.tile([1, NS], F32, tag="ss2")
                    for dct in range(cfg.NDCT):
                        wo = p3.tile([P, TD // P, P], F32R, tag="wo")
                        nc.sync.dma_start(out=wo[:], in_=wo_t[dct])
                        ops = p3ps.tile([P, NS], F32, tag="ops")
                        for tdt in range(TD // P):
                            nc.tensor.matmul(ops[:], (wo[:, tdt, :]),
                                             (ot_sb[:, tdt, :]),
                                             start=(tdt == 0),
                                             stop=(tdt == TD // P - 1))
                        xq = p3.tile([P, NS], F32, tag="xq")
                        nc.sync.dma_start(
                            out=xq[:],
                            in_=xTq_t[dct, :, ts * NS:(ts + 1) * NS])
                        x2 = p3.tile([P, NS], F32, tag="x2")
                        nc.vector.tensor_tensor(x2[:], ops[:], xq[:], ALU.add)
                        nc.sync.dma_start(
                            out=x2T_d[dct, :, ts * NS:(ts + 1) * NS],
                            in_=x2[:])
                        sq2 = p3.tile([P, NS], F32R, tag="sq2")
                        nc.scalar.activation(sq2[:], x2[:], AF.Square)
                        nc.tensor.matmul(ssp2[:], (ones[:]), (sq2[:]),
                                         start=(dct == 0),
                                         stop=(dct == cfg.NDCT - 1))
                    srow2 = p3.tile([1, NS], F32, tag="sr2")
                    nc.scalar.activation(srow2[:], ssp2[:], AF.Sqrt,
                                         scale=1.0 / D, bias=epsT[:])
                    nc.vector.reciprocal(rec3[:, ts * NS:(ts + 1) * NS],
                                         srow2[:])
                nc.gpsimd.partition_broadcast(bc2[:], rec3[:1, :])

            # ============ P4+P5: h2 + SwiGLU MLP + residual ============
            with tc.tile_pool(name="pres", bufs=1) as pres, \
                 tc.tile_pool(name="p5", bufs=2) as p5, \
                 tc.tile_pool(name="p5s", bufs=2) as p5s, \
                 tc.tile_pool(name="p5ps", bufs=2, space="PSUM") as p5ps:
                h2T = pres.tile([P, DT, QT], F32R, tag="h2T")
                y_acc = pres.tile([P, DT, QT], F32, tag="y_acc")
                for dt in range(DT):
                    x2t = p5.tile([P, QT], F32, tag="x2t")
                    nc.sync.dma_start(out=x2t[:], in_=x2T_d[dt])
                    nc.vector.tensor_tensor(h2T[:, dt, :], x2t[:], bc2[:],
                                            ALU.mult)
                    nc.scalar.copy(y_acc[:, dt, :], x2t[:])
                for sp in range(cfg.NSP):
                    mt = p5s.tile([P, cfg.FT_SP, QT], F32R, tag="mt", bufs=1)
                    for ft in range(cfg.FT_SP):
                        gft = sp * cfg.FT_SP + ft
                        wg = p5.tile([P, DT, P], F32R, tag="wg")
                        nc.sync.dma_start(out=wg[:], in_=wg_t[gft])
                        wu = p5.tile([P, DT, P], F32R, tag="wu")
                        nc.sync.dma_start(out=wu[:], in_=wu_t[gft])
                        for ws in range(cfg.NQS):
                            gps = p5ps.tile([P, NS], F32, tag="gps")
                            for dt in range(DT):
                                nc.tensor.matmul(
                                    gps[:], (wg[:, dt, :]),
                                    (h2T[:, dt, ws * NS:(ws + 1) * NS]),
                                    start=(dt == 0), stop=(dt == DT - 1))
                            ups = p5ps.tile([P, NS], F32, tag="ups")
                            for dt in range(DT):
                                nc.tensor.matmul(
                                    ups[:], (wu[:, dt, :]),
                                    (h2T[:, dt, ws * NS:(ws + 1) * NS]),
                                    start=(dt == 0), stop=(dt == DT - 1))
                            sg = p5.tile([P, NS], F32, tag="sg")
                            nc.scalar.activation(sg[:], gps[:], AF.Silu)
                            nc.vector.tensor_tensor(
                                mt[:, ft, ws * NS:(ws + 1) * NS], sg[:],
                                ups[:], ALU.mult)
                    for dct in range(cfg.NDCT):
                        wd = p5.tile([P, cfg.FT_SP, P], F32R, tag="wd")
                        nc.sync.dma_start(out=wd[:], in_=wd_t[sp, dct])
                        for ws in range(cfg.NQS):
                            dps = p5ps.tile([P, NS], F32, tag="dps")
                            for ft in range(cfg.FT_SP):
                                nc.tensor.matmul(
                                    dps[:], (wd[:, ft, :]),
                                    (mt[:, ft, ws * NS:(ws + 1) * NS]),
                                    start=(ft == 0),
                                    stop=(ft == cfg.FT_SP - 1))
                            ya = y_acc[:, dct, ws * NS:(ws + 1) * NS]
                            nc.vector.tensor_tensor(ya, ya, dps[:], ALU.add)
                for dt in range(DT):
                    nc.sync.dma_start(out=yT[dt * P:(dt + 1) * P, :],
                                      in_=y_acc[:, dt, :])

    nc.compile()
    return nc


# --------------------------------------------------------------------------
# Host side
# --------------------------------------------------------------------------

_NC_CACHE = {}


def _get_nc(cfg):
    key = (cfg.D, cfg.T, cfg.TD, cfg.FF, cfg.QT, cfg.NS)
    if key not in _NC_CACHE:
        _NC_CACHE[key] = build(cfg)
    return _NC_CACHE[key]


def prep_weights(cfg, w_qkv, w_out, w_gate, w_up, w_down, ln1, ln2):
    D, TD, FF, NS = cfg.D, cfg.TD, cfg.FF, cfg.NS
    DT = cfg.DT
    f32 = np.float32
    w_qkv_f = (np.asarray(w_qkv, f32) * np.asarray(ln1, f32)[None, :])
    wqT = w_qkv_f[0:TD].T
    wkT = w_qkv_f[TD:2 * TD].T
    wvT = w_qkv_f[2 * TD:3 * TD].T
    woT = np.asarray(w_out, f32).T            # [TD, D]
    wgT = (np.asarray(w_gate, f32) * np.asarray(ln2, f32)[None, :]).T
    wuT = (np.asarray(w_up, f32) * np.asarray(ln2, f32)[None, :]).T
    wdT = np.asarray(w_down, f32).T           # [FF, D]

    def tile_lhs(a, ncols):  # [D, C] -> [C/ncols? ...] lhsT tiles [ct, P, dt, cP]
        d, c = a.shape
        return np.ascontiguousarray(
            a.reshape(d // P, P, c // ncols, ncols).transpose(2, 1, 0, 3))

    wq_t = tile_lhs(wqT, P)
    wk_t = tile_lhs(wkT, P)
    wv_t = tile_lhs(wvT, NS)
    wo_t = tile_lhs(woT, P)
    wg_t = tile_lhs(wgT, P)
    wu_t = tile_lhs(wuT, P)
    # w_down: [FF, D] -> [sp, dct, P, ft, P]
    wd_t = np.ascontiguousarray(
        wdT.reshape(cfg.NSP, cfg.FT_SP, P, D // P, P).transpose(0, 3, 2, 1, 4))
    return dict(wq_t=wq_t, wk_t=wk_t, wv_t=wv_t, wo_t=wo_t, wg_t=wg_t,
                wu_t=wu_t, wd_t=wd_t)


def prep_core_inputs(cfg, xb, parity, wdict):
    """Per-core tensors for batch slice xb [T, D]; query tokens are the
    interleaved slice parity::stride (stride = T // QT)."""
    T, D, QT, KT = cfg.T, cfg.D, cfg.QT, cfg.KT
    stride = T // QT
    f32 = np.float32
    xT = np.ascontiguousarray(np.asarray(xb, f32).T)        # [D, T]
    xT_t = xT.reshape(cfg.DT, P, T)
    xTq_t = np.ascontiguousarray(xT[:, parity::stride]).reshape(cfg.DT, P, QT)
    kk = np.arange(T)[:, None]
    qq = np.arange(QT)[None, :] * stride + parity
    m = np.where(kk <= qq, 0.0, -1e30).astype(f32)          # [T, QT]
    mask_t = np.ascontiguousarray(m.reshape(KT, P, QT).transpose(1, 0, 2))
    out = dict(xT_t=xT_t, xTq_t=xTq_t, mask_t=mask_t,
               ones_in=np.ones((P, 1), np.float32))
    out.update(wdict)
    return out


def run(cfg, x, w_qkv, w_out, w_gate, w_up, w_down, ln1, ln2):
    nc = _get_nc(cfg)
    wdict = prep_weights(cfg, w_qkv, w_out, w_gate, w_up, w_down, ln1, ln2)
    x = np.asarray(x, np.float32)
    Bc = x.shape[0]
    in_maps = []
    for c in range(N_CORES):
        b, half = divmod(c, 2)
        b = b % Bc
        in_maps.append(prep_core_inputs(cfg, x[b], half, wdict))
    res = run_bass_kernel_spmd(nc, in_maps, list(range(N_CORES)))
    stride = cfg.T // cfg.QT
    y = np.empty((Bc, cfg.T, cfg.D), np.float32)
    for c in range(N_CORES):
        b, parity = divmod(c, 2)
        if b < Bc:
            y[b, parity::stride, :] = res.results[c]["yT"].T
    return y


def kernel(x, w_qkv, w_out, w_gate, w_up, w_down, ln1, ln2):
    return run(FULL, x, w_qkv, w_out, w_gate, w_up, w_down, ln1, ln2)
